# revision 10
# baseline (speedup 1.0000x reference)
"""Binary 3-layer CNN (sign activations + sign weights) on 8 NeuronCores.

Strategy: pure data parallel — 64 images -> 8 cores x 8 images, fp8 compute.
All matmul operands are exactly +-1/0 -> fp8e4m3 with fp32 PSUM accumulation
is numerically exact.

Per core, 2 batches of 4 images; SBUF partition layout [128 = (4 img, 32 ch)].
The three convs are FUSED per strip of R output rows with halo recompute
(strip s computes R+4 rows of h0 -> R+2 rows of h1 -> R output rows), so h0
and h1 never leave SBUF; only the padded sign(x) plane stages through DRAM.

 - conv0 (1->32ch): all 9 taps packed into K: input replicated into 36
   partitions (tap, img) via 9 shifted DMAs from the extended-pad sign(x)
   plane; one matmul per output row-pair (K=36, M=128, N=512).
 - conv1 (32->32ch): fp8 DoubleRow matmuls, 2 taps per pass: K=128 partitions
   x 2 k-subtiles; the rhs k-subtile offset is a free-dim shift on the padded
   input tile (4D AP), so 9 taps cost 5 passes instead of 9. Tap pairs share
   one (row,col) shift delta; the odd 9th tap is paired with zero weights.
 - conv2 (32->1ch): same DoubleRow pairing, M=4 (one column per image),
   psum [4, 8, 256] -> bf16 output (conv2 sums are even integers <= 288,
   exact in bf16).
PSUM is used as [128, 8, 256] 4-bank supertiles (4 row-pair matmul groups +
one batched eviction). sign() eviction: psum -> fp8 via ScalarE Sign (conv0)
and VectorE clamp(-1,1) (conv1; sums are integers so clamp == sign).
Issue order is software-pipelined (A of strip s+1 before B of strip s) to
hide eviction tails from the in-order PE queue.
"""

import numpy as np
import ml_dtypes

import concourse.bass as bass
import concourse.mybir as mybir
import concourse.tile as tile
from concourse import bacc
from concourse.bass_utils import run_bass_kernel_spmd

FP8 = mybir.dt.float8e4
BF16 = mybir.dt.bfloat16
F32 = mybir.dt.float32
AF = mybir.ActivationFunctionType
ALU = mybir.AluOpType
DRM = mybir.MatmulPerfMode.DoubleRow

N_CORES = 8
IMG_PER_CORE = 8
B = 4          # images per partition-batch
H = W = 256
WP = 258       # padded width (1 col pad each side)
HE = 262       # extended padded height: row = x row + 3
R = 64         # strip rows (output rows per strip)
NS = H // R    # strips per batch
NB = IMG_PER_CORE // B  # batches per core

# DoubleRow tap pairs: both taps of a pair share one flat shift delta
# (dy*WP + dx); the 9th tap is paired with zero weights (k-slot 1 unused).
PAIRS = [
    ((0, 0), (0, 1)),
    ((1, 0), (1, 1)),
    ((2, 0), (2, 1)),
    ((0, 2), (1, 2)),
    ((2, 2), None),
]


def _dr_rhs(hin, r, pair):
    """4D DoubleRow rhs AP: [128, ksub=2, rows=2, cols=256] with the ksub
    dim stepping by the tap-pair's shift delta over the padded tile."""
    (dy0, dx0), t1 = pair
    # the zero-weight dummy slot points one row up: always inside the tile
    delta = -WP if t1 is None else (t1[0] - dy0) * WP + (t1[1] - dx0)
    sl = hin[:, r + dy0:r + dy0 + 2, dx0:dx0 + 256]
    return bass.AP(
        tensor=sl.tensor, offset=sl.offset,
        ap=[list(sl.ap[0]), [delta, 2], list(sl.ap[1]), list(sl.ap[2])])


def _build_program(stages=('0', 'A', 'B', 'C')):
    nc = bacc.Bacc("TRN2", target_bir_lowering=False, debug=False)

    x_in = nc.dram_tensor("x", [IMG_PER_CORE, H, W], F32, kind="ExternalInput")
    s0_in = nc.dram_tensor("s0", [36, 128], FP8, kind="ExternalInput")
    s1_in = nc.dram_tensor("s1", [128, 5, 2, 128], FP8, kind="ExternalInput")
    s2_in = nc.dram_tensor("s2", [128, 5, 2, 16], FP8, kind="ExternalInput")
    out_d = nc.dram_tensor("out", [IMG_PER_CORE, H, W], BF16,
                           kind="ExternalOutput")

    # extended-pad sign(x): row = x row + 3 (rows 0-2 and 259-261 zero)
    xs_d = nc.dram_tensor("xs", [IMG_PER_CORE, HE, WP], FP8)

    with tile.TileContext(nc) as tc:
        with (
            tc.tile_pool(name="const", bufs=1) as cpool,
            tc.tile_pool(name="xprep", bufs=4) as xpool,
            tc.tile_pool(name="xrep", bufs=2) as xrpool,
            tc.tile_pool(name="h0", bufs=2) as h0pool,
            tc.tile_pool(name="h1", bufs=2) as h1pool,
            tc.tile_pool(name="cout", bufs=2) as cpool2,
            tc.tile_pool(name="psum", bufs=2, space="PSUM") as pspool,
        ):
            # --- constants: stationary weights + a zero tile ---
            s0t = cpool.tile([36, 128], FP8, tag="s0")
            nc.sync.dma_start(out=s0t[:, :], in_=s0_in[:, :])
            s1t = cpool.tile([128, 5, 2, 128], FP8, tag="s1")
            nc.sync.dma_start(out=s1t[:, :, :, :], in_=s1_in[:, :, :, :])
            s2t = cpool.tile([128, 5, 2, 16], FP8, tag="s2")
            nc.sync.dma_start(out=s2t[:, :, :, :], in_=s2_in[:, :, :, :])
            zt = cpool.tile([128, WP], FP8, tag="zt")
            nc.gpsimd.memset(zt[:, :], 0.0)

            # --- pre-zero xs pad rows (cols are baked into SBUF tiles) ---
            for img in range(IMG_PER_CORE):
                nc.scalar.dma_start(out=xs_d[img, 0:3, :], in_=zt[0:3, :])
                nc.scalar.dma_start(out=xs_d[img, HE - 3:HE, :],
                                    in_=zt[0:3, :])

            # --- stage 0: sign(x) -> extended-pad fp8 planes in DRAM ---
            for img in range(IMG_PER_CORE if '0' in stages else 0):
                for rb in range(H // 128):
                    xf = xpool.tile([128, W], F32, tag="xf")
                    nc.sync.dma_start(
                        out=xf[:, :], in_=x_in[img, rb * 128:(rb + 1) * 128, :])
                    xp = xpool.tile([128, WP], FP8, tag="xp")
                    nc.scalar.activation(xp[:, 1:W + 1], xf[:, :], AF.Sign)
                    nc.gpsimd.memset(xp[:, 0:1], 0.0)
                    nc.gpsimd.memset(xp[:, WP - 1:WP], 0.0)
                    nc.scalar.dma_start(
                        out=xs_d[img, rb * 128 + 3:(rb + 1) * 128 + 3, :],
                        in_=xp[:, :])

            for b in range(NB):
                ht0s, ht1s = {}, {}

                def stage_a(s, b=b):
                    """conv0 strip s: h0 rows [sR-2, sR+R+2) -> ht0 tile
                    (tile row i = h0 row sR-2+i)."""
                    xt = xrpool.tile([36, R + 4, 256], FP8, tag="xrep",
                                     name="xt")
                    for dy in range(3):
                        for dx in range(3):
                            t = dy * 3 + dx
                            nc.sync.dma_start(
                                out=xt[4 * t:4 * t + 4, :, :],
                                in_=xs_d[b * B:(b + 1) * B,
                                         s * R + dy:s * R + dy + R + 4,
                                         dx:dx + 256])
                    ht0 = h0pool.tile([128, R + 4, WP], FP8, tag="h0",
                                      name="ht0")
                    nc.gpsimd.memset(ht0[:, :, 0:1], 0.0)
                    nc.gpsimd.memset(ht0[:, :, WP - 1:WP], 0.0)
                    for r0 in range(0, R + 4, 8):
                        nrow = min(8, R + 4 - r0)
                        ps = pspool.tile([128, 8, 256], F32, tag="ps",
                                         name="psA")
                        for q in range(nrow // 2):
                            nc.tensor.matmul(
                                ps[:, 2 * q:2 * q + 2, :], s0t[:, :],
                                xt[:, r0 + 2 * q:r0 + 2 * q + 2, :],
                                start=True, stop=True)
                        nc.scalar.activation(
                            ht0[:, r0:r0 + nrow, 1:W + 1],
                            ps[:, 0:nrow, :], AF.Sign)
                    # boundary: h0 pad rows (-1 / 256) must be zero
                    if s == 0:
                        nc.gpsimd.memset(ht0[:, 1:2, :], 0.0)
                    if s == NS - 1:
                        nc.gpsimd.memset(ht0[:, R + 2:R + 3, :], 0.0)
                    return ht0

                def stage_b(s, ht0):
                    """conv1 strip s: h1 rows [sR-1, sR+R+1) -> ht1 tile
                    (tile row i = h1 row sR-1+i); input ht0."""
                    ht1 = h1pool.tile([128, R + 2, WP], FP8, tag="h1",
                                      name="ht1")
                    nc.gpsimd.memset(ht1[:, :, 0:1], 0.0)
                    nc.gpsimd.memset(ht1[:, :, WP - 1:WP], 0.0)
                    for r0 in range(0, R + 2, 8):
                        nrow = min(8, R + 2 - r0)
                        ps = pspool.tile([128, 8, 256], F32, tag="ps",
                                         name="psB")
                        for q in range(nrow // 2):
                            for t, pair in enumerate(PAIRS):
                                nc.tensor.matmul(
                                    ps[:, 2 * q:2 * q + 2, :], s1t[:, t, :, :],
                                    _dr_rhs(ht0, r0 + 2 * q, pair),
                                    start=(t == 0), stop=(t == 4),
                                    perf_mode=DRM)
                        nc.vector.tensor_scalar(
                            ht1[:, r0:r0 + nrow, 1:W + 1], ps[:, 0:nrow, :],
                            -1.0, 1.0, ALU.max, ALU.min)
                    # boundary: h1 pad rows (-1 / 256) must be zero
                    if s == 0:
                        nc.gpsimd.memset(ht1[:, 0:1, :], 0.0)
                    if s == NS - 1:
                        nc.gpsimd.memset(ht1[:, R + 1:R + 2, :], 0.0)
                    return ht1

                def stage_c(s, ht1, b=b):
                    """conv2 strip s: out rows [sR, sR+R); input ht1."""
                    ot = cpool2.tile([B, R, W], BF16, tag="c_out", name="ot")
                    for r0 in range(0, R, 8):
                        ps = pspool.tile([128, 8, 256], F32, tag="ps",
                                         name="psC")
                        for q in range(4):
                            for t, pair in enumerate(PAIRS):
                                nc.tensor.matmul(
                                    ps[0:B, 2 * q:2 * q + 2, :],
                                    s2t[:, t, :, 0:B],
                                    _dr_rhs(ht1, r0 + 2 * q, pair),
                                    start=(t == 0), stop=(t == 4),
                                    perf_mode=DRM)
                        if (r0 // 8) % 2 == 0:
                            nc.scalar.activation(
                                ot[:, r0:r0 + 8, :], ps[0:B, :, :], AF.Copy)
                        else:
                            nc.vector.tensor_copy(
                                ot[:, r0:r0 + 8, :], ps[0:B, :, :])
                    for g in range(B):
                        nc.gpsimd.dma_start(
                            out=out_d[b * B + g, s * R:s * R + R, :],
                            in_=ot[g:g + 1, :, :])

                # software pipeline, depth 2: issue A(s), B(s-1), C(s-2) so
                # the in-order PE never waits on an eviction tail.
                for s in range(NS + 2):
                    if s < NS and 'A' in stages:
                        ht0s[s] = stage_a(s)
                    if 0 <= s - 1 < NS and 'B' in stages and s - 1 in ht0s:
                        ht1s[s - 1] = stage_b(s - 1, ht0s.pop(s - 1))
                    if 0 <= s - 2 < NS and 'C' in stages and s - 2 in ht1s:
                        stage_c(s - 2, ht1s.pop(s - 2))
    nc.compile()
    return nc


def _host_weights(w0, w1, w2):
    """Pack sign(w) into fp8 stationary matrices (see module docstring)."""
    f8 = ml_dtypes.float8_e4m3
    sg = lambda w: np.sign(np.asarray(w, np.float32))
    w0s, w1s, w2s = sg(w0), sg(w1), sg(w2)  # [32,1,3,3],[32,32,3,3],[1,32,3,3]
    s0 = np.zeros((36, 128), np.float32)
    s1 = np.zeros((128, 5, 2, 128), np.float32)
    s2 = np.zeros((128, 5, 2, 16), np.float32)
    for g in range(B):
        for dy in range(3):
            for dx in range(3):
                t = dy * 3 + dx
                s0[t * 4 + g, g * 32:(g + 1) * 32] = w0s[:, 0, dy, dx]
        for t, ((dy0, dx0), t1) in enumerate(PAIRS):
            s1[g * 32:(g + 1) * 32, t, 0, g * 32:(g + 1) * 32] = \
                w1s[:, :, dy0, dx0].T  # [ci, co]
            s2[g * 32:(g + 1) * 32, t, 0, g] = w2s[0, :, dy0, dx0]
            if t1 is not None:
                s1[g * 32:(g + 1) * 32, t, 1, g * 32:(g + 1) * 32] = \
                    w1s[:, :, t1[0], t1[1]].T
                s2[g * 32:(g + 1) * 32, t, 1, g] = w2s[0, :, t1[0], t1[1]]
    return s0.astype(f8), s1.astype(f8), s2.astype(f8)


_NC_CACHE = {}


def kernel(x, w0, w1, w2):
    if "nc" not in _NC_CACHE:
        _NC_CACHE["nc"] = _build_program()
    nc = _NC_CACHE["nc"]
    s0, s1, s2 = _host_weights(w0, w1, w2)
    x = np.asarray(x, np.float32).reshape(64, H, W)
    in_maps = [
        {"x": np.ascontiguousarray(x[i * IMG_PER_CORE:(i + 1) * IMG_PER_CORE]),
         "s0": s0, "s1": s1, "s2": s2}
        for i in range(N_CORES)
    ]
    res = run_bass_kernel_spmd(nc, in_maps, list(range(N_CORES)))
    out = np.stack([np.asarray(res.results[i]["out"], np.float32)
                    for i in range(N_CORES)])
    return out.reshape(64, 1, H, W)


# revision 11
# speedup vs baseline: 1.1239x; 1.1239x over previous
"""Binary 3-layer CNN (sign activations + sign weights) on 8 NeuronCores.

Strategy: pure data parallel — 64 images -> 8 cores x 8 images, fp8 compute.
All matmul operands are exactly +-1/0 -> fp8e4m3 with fp32 PSUM accumulation
is numerically exact.

Per core, 2 batches of 4 images; SBUF partition layout [128 = (4 img, 32 ch)].
The three convs are FUSED per strip of R output rows with halo recompute
(strip s computes R+4 rows of h0 -> R+2 rows of h1 -> R output rows), so h0
and h1 never leave SBUF; only the padded sign(x) plane stages through DRAM.

 - conv0 (1->32ch): all 9 taps packed into K: input replicated into 36
   partitions (tap, img) via 9 shifted DMAs from the extended-pad sign(x)
   plane; one matmul per output row-pair (K=36, M=128, N=512).
 - conv1 (32->32ch): fp8 DoubleRow matmuls, 2 taps per pass: K=128 partitions
   x 2 k-subtiles; the rhs k-subtile offset is a free-dim shift on the padded
   input tile (4D AP), so 9 taps cost 5 passes instead of 9. Tap pairs share
   one (row,col) shift delta; the odd 9th tap is paired with zero weights.
 - conv2 (32->1ch): same DoubleRow pairing, M=4 (one column per image),
   psum [4, 8, 256] -> bf16 output (conv2 sums are even integers <= 288,
   exact in bf16).
PSUM is used as [128, 8, 256] 4-bank supertiles (4 row-pair matmul groups +
one batched eviction). sign() eviction: psum -> fp8 via ScalarE Sign (conv0)
and VectorE clamp(-1,1) (conv1; sums are integers so clamp == sign).
Issue order is software-pipelined (A of strip s+1 before B of strip s) to
hide eviction tails from the in-order PE queue.
"""

import numpy as np
import ml_dtypes

import concourse.bass as bass
import concourse.mybir as mybir
import concourse.tile as tile
from concourse import bacc
from concourse.bass_utils import run_bass_kernel_spmd

FP8 = mybir.dt.float8e4
BF16 = mybir.dt.bfloat16
F32 = mybir.dt.float32
AF = mybir.ActivationFunctionType
ALU = mybir.AluOpType
DRM = mybir.MatmulPerfMode.DoubleRow

N_CORES = 8
IMG_PER_CORE = 8
B = 4          # images per partition-batch
H = W = 256
WP = 258       # padded width (1 col pad each side)
HE = 262       # extended padded height: row = x row + 3
R = 64         # strip rows (output rows per strip)
NS = H // R    # strips per batch
NB = IMG_PER_CORE // B  # batches per core

# DoubleRow tap pairs: both taps of a pair share one flat shift delta
# (dy*WP + dx); the 9th tap is paired with zero weights (k-slot 1 unused).
PAIRS = [
    ((0, 0), (0, 1)),
    ((1, 0), (1, 1)),
    ((2, 0), (2, 1)),
    ((0, 2), (1, 2)),
    ((2, 2), None),
]


def _dr_rhs(hin, r, pair):
    """4D DoubleRow rhs AP: [128, ksub=2, rows=2, cols=256] with the ksub
    dim stepping by the tap-pair's shift delta over the padded tile."""
    (dy0, dx0), t1 = pair
    # the zero-weight dummy slot points one row up: always inside the tile
    delta = -WP if t1 is None else (t1[0] - dy0) * WP + (t1[1] - dx0)
    sl = hin[:, r + dy0:r + dy0 + 2, dx0:dx0 + 256]
    return bass.AP(
        tensor=sl.tensor, offset=sl.offset,
        ap=[list(sl.ap[0]), [delta, 2], list(sl.ap[1]), list(sl.ap[2])])


def _build_program(stages=('0', 'A', 'B', 'C')):
    nc = bacc.Bacc("TRN2", target_bir_lowering=False, debug=False)

    x_in = nc.dram_tensor("x", [IMG_PER_CORE, H, W], F32, kind="ExternalInput")
    s0_in = nc.dram_tensor("s0", [36, 128], FP8, kind="ExternalInput")
    s1_in = nc.dram_tensor("s1", [128, 5, 2, 128], FP8, kind="ExternalInput")
    s2_in = nc.dram_tensor("s2", [128, 5, 2, 16], FP8, kind="ExternalInput")
    out_d = nc.dram_tensor("out", [IMG_PER_CORE, H, W], BF16,
                           kind="ExternalOutput")

    # extended-pad sign(x): row = x row + 3 (rows 0-2 and 259-261 zero)
    xs_d = nc.dram_tensor("xs", [IMG_PER_CORE, HE, WP], FP8)

    with tile.TileContext(nc) as tc:
        with (
            tc.tile_pool(name="const", bufs=1) as cpool,
            tc.tile_pool(name="xprep", bufs=4) as xpool,
            tc.tile_pool(name="xrep", bufs=2) as xrpool,
            tc.tile_pool(name="h0", bufs=2) as h0pool,
            tc.tile_pool(name="h1", bufs=2) as h1pool,
            tc.tile_pool(name="cout", bufs=2) as cpool2,
            tc.tile_pool(name="psum", bufs=2, space="PSUM") as pspool,
        ):
            # --- constants: stationary weights + a zero tile ---
            s0t = cpool.tile([36, 128], FP8, tag="s0")
            nc.sync.dma_start(out=s0t[:, :], in_=s0_in[:, :])
            s1t = cpool.tile([128, 5, 2, 128], FP8, tag="s1")
            nc.sync.dma_start(out=s1t[:, :, :, :], in_=s1_in[:, :, :, :])
            s2t = cpool.tile([128, 5, 2, 16], FP8, tag="s2")
            nc.sync.dma_start(out=s2t[:, :, :, :], in_=s2_in[:, :, :, :])
            zt = cpool.tile([128, WP], FP8, tag="zt")
            nc.gpsimd.memset(zt[:, :], 0.0)

            # --- pre-zero xs pad rows (cols are baked into SBUF tiles) ---
            for img in range(IMG_PER_CORE):
                nc.scalar.dma_start(out=xs_d[img, 0:3, :], in_=zt[0:3, :])
                nc.scalar.dma_start(out=xs_d[img, HE - 3:HE, :],
                                    in_=zt[0:3, :])

            # --- stage 0: sign(x) -> extended-pad fp8 planes in DRAM ---
            for img in range(IMG_PER_CORE if '0' in stages else 0):
                for rb in range(H // 128):
                    xf = xpool.tile([128, W], F32, tag="xf")
                    nc.sync.dma_start(
                        out=xf[:, :], in_=x_in[img, rb * 128:(rb + 1) * 128, :])
                    xp = xpool.tile([128, WP], FP8, tag="xp")
                    nc.scalar.activation(xp[:, 1:W + 1], xf[:, :], AF.Sign)
                    nc.gpsimd.memset(xp[:, 0:1], 0.0)
                    nc.gpsimd.memset(xp[:, WP - 1:WP], 0.0)
                    nc.scalar.dma_start(
                        out=xs_d[img, rb * 128 + 3:(rb + 1) * 128 + 3, :],
                        in_=xp[:, :])

            for b in range(NB):
                ht0s, ht1s = {}, {}

                def stage_a_units(s, b=b):
                    """conv0 strip s: h0 rows [sR-2, sR+R+2) -> ht0 tile
                    (tile row i = h0 row sR-2+i). Returns (ht0, units)."""
                    xt = xrpool.tile([36, R + 4, 256], FP8, tag="xrep",
                                     name="xt")
                    for dy in range(3):
                        for dx in range(3):
                            t = dy * 3 + dx
                            nc.sync.dma_start(
                                out=xt[4 * t:4 * t + 4, :, :],
                                in_=xs_d[b * B:(b + 1) * B,
                                         s * R + dy:s * R + dy + R + 4,
                                         dx:dx + 256])
                    ht0 = h0pool.tile([128, R + 4, WP], FP8, tag="h0",
                                      name="ht0")
                    nc.gpsimd.memset(ht0[:, :, 0:1], 0.0)
                    nc.gpsimd.memset(ht0[:, :, WP - 1:WP], 0.0)

                    def unit(r0, last):
                        nrow = min(8, R + 4 - r0)
                        ps = pspool.tile([128, 8, 256], F32, tag="ps",
                                         name="psA")
                        for q in range(nrow // 2):
                            nc.tensor.matmul(
                                ps[:, 2 * q:2 * q + 2, :], s0t[:, :],
                                xt[:, r0 + 2 * q:r0 + 2 * q + 2, :],
                                start=True, stop=True)
                        nc.scalar.activation(
                            ht0[:, r0:r0 + nrow, 1:W + 1],
                            ps[:, 0:nrow, :], AF.Sign)
                        if last:
                            # boundary: h0 pad rows (-1 / 256) must be zero
                            if s == 0:
                                nc.gpsimd.memset(ht0[:, 1:2, :], 0.0)
                            if s == NS - 1:
                                nc.gpsimd.memset(ht0[:, R + 2:R + 3, :], 0.0)

                    r0s = list(range(0, R + 4, 8))
                    return ht0, [
                        (lambda r0=r0, last=(r0 == r0s[-1]): unit(r0, last))
                        for r0 in r0s]

                def stage_b_units(s, ht0):
                    """conv1 strip s: h1 rows [sR-1, sR+R+1) -> ht1 tile
                    (tile row i = h1 row sR-1+i); input ht0."""
                    ht1 = h1pool.tile([128, R + 2, WP], FP8, tag="h1",
                                      name="ht1")
                    nc.gpsimd.memset(ht1[:, :, 0:1], 0.0)
                    nc.gpsimd.memset(ht1[:, :, WP - 1:WP], 0.0)

                    def unit(r0, last):
                        nrow = min(8, R + 2 - r0)
                        ps = pspool.tile([128, 8, 256], F32, tag="ps",
                                         name="psB")
                        for q in range(nrow // 2):
                            for t, pair in enumerate(PAIRS):
                                nc.tensor.matmul(
                                    ps[:, 2 * q:2 * q + 2, :], s1t[:, t, :, :],
                                    _dr_rhs(ht0, r0 + 2 * q, pair),
                                    start=(t == 0), stop=(t == 4),
                                    perf_mode=DRM)
                        nc.scalar.activation(
                            ht1[:, r0:r0 + nrow, 1:W + 1], ps[:, 0:nrow, :],
                            AF.Sign)
                        if last:
                            # boundary: h1 pad rows (-1 / 256) must be zero
                            if s == 0:
                                nc.gpsimd.memset(ht1[:, 0:1, :], 0.0)
                            if s == NS - 1:
                                nc.gpsimd.memset(ht1[:, R + 1:R + 2, :], 0.0)

                    r0s = list(range(0, R + 2, 8))
                    return ht1, [
                        (lambda r0=r0, last=(r0 == r0s[-1]): unit(r0, last))
                        for r0 in r0s]

                def stage_c_units(s, ht1, b=b):
                    """conv2 strip s: out rows [sR, sR+R); input ht1."""
                    ot = cpool2.tile([B, R, W], BF16, tag="c_out", name="ot")

                    def unit(r0, last):
                        ps = pspool.tile([128, 8, 256], F32, tag="ps",
                                         name="psC")
                        for q in range(4):
                            for t, pair in enumerate(PAIRS):
                                nc.tensor.matmul(
                                    ps[0:B, 2 * q:2 * q + 2, :],
                                    s2t[:, t, :, 0:B],
                                    _dr_rhs(ht1, r0 + 2 * q, pair),
                                    start=(t == 0), stop=(t == 4),
                                    perf_mode=DRM)
                        nc.vector.tensor_copy(
                            ot[:, r0:r0 + 8, :], ps[0:B, :, :])
                        if last:
                            for g in range(B):
                                nc.gpsimd.dma_start(
                                    out=out_d[b * B + g, s * R:s * R + R, :],
                                    in_=ot[g:g + 1, :, :])

                    r0s = list(range(0, R, 8))
                    return [
                        (lambda r0=r0, last=(r0 == r0s[-1]): unit(r0, last))
                        for r0 in r0s]

                # software pipeline, depth 2, interleaved at supertile
                # granularity: fast-matmul A units ride alongside slow-matmul
                # B/C units so the in-order PE never drains the 2-buffer PSUM
                # rotation waiting on an eviction.
                for s in range(NS + 2):
                    units = []
                    if s < NS and 'A' in stages:
                        ht0s[s], ua = stage_a_units(s)
                        units.append(ua)
                    if 0 <= s - 1 < NS and 'B' in stages and s - 1 in ht0s:
                        ht1s[s - 1], ub = stage_b_units(s - 1,
                                                        ht0s.pop(s - 1))
                        units.append(ub)
                    if 0 <= s - 2 < NS and 'C' in stages and s - 2 in ht1s:
                        units.append(stage_c_units(s - 2, ht1s.pop(s - 2)))
                    for i in range(max(map(len, units), default=0)):
                        for u in units:
                            if i < len(u):
                                u[i]()
    nc.compile()
    return nc


def _host_weights(w0, w1, w2):
    """Pack sign(w) into fp8 stationary matrices (see module docstring)."""
    f8 = ml_dtypes.float8_e4m3
    sg = lambda w: np.sign(np.asarray(w, np.float32))
    w0s, w1s, w2s = sg(w0), sg(w1), sg(w2)  # [32,1,3,3],[32,32,3,3],[1,32,3,3]
    s0 = np.zeros((36, 128), np.float32)
    s1 = np.zeros((128, 5, 2, 128), np.float32)
    s2 = np.zeros((128, 5, 2, 16), np.float32)
    for g in range(B):
        for dy in range(3):
            for dx in range(3):
                t = dy * 3 + dx
                s0[t * 4 + g, g * 32:(g + 1) * 32] = w0s[:, 0, dy, dx]
        for t, ((dy0, dx0), t1) in enumerate(PAIRS):
            s1[g * 32:(g + 1) * 32, t, 0, g * 32:(g + 1) * 32] = \
                w1s[:, :, dy0, dx0].T  # [ci, co]
            s2[g * 32:(g + 1) * 32, t, 0, g] = w2s[0, :, dy0, dx0]
            if t1 is not None:
                s1[g * 32:(g + 1) * 32, t, 1, g * 32:(g + 1) * 32] = \
                    w1s[:, :, t1[0], t1[1]].T
                s2[g * 32:(g + 1) * 32, t, 1, g] = w2s[0, :, t1[0], t1[1]]
    return s0.astype(f8), s1.astype(f8), s2.astype(f8)


_NC_CACHE = {}


def kernel(x, w0, w1, w2):
    if "nc" not in _NC_CACHE:
        _NC_CACHE["nc"] = _build_program()
    nc = _NC_CACHE["nc"]
    s0, s1, s2 = _host_weights(w0, w1, w2)
    x = np.asarray(x, np.float32).reshape(64, H, W)
    in_maps = [
        {"x": np.ascontiguousarray(x[i * IMG_PER_CORE:(i + 1) * IMG_PER_CORE]),
         "s0": s0, "s1": s1, "s2": s2}
        for i in range(N_CORES)
    ]
    res = run_bass_kernel_spmd(nc, in_maps, list(range(N_CORES)))
    out = np.stack([np.asarray(res.results[i]["out"], np.float32)
                    for i in range(N_CORES)])
    return out.reshape(64, 1, H, W)


# revision 13
# speedup vs baseline: 2.0502x; 1.8242x over previous
"""Binary 3-layer CNN (sign activations + sign weights) on 8 NeuronCores.

Strategy: pure data parallel — 64 images -> 8 cores x 8 images, fp8 compute.
All matmul operands are exactly +-1/0 -> fp8e4m3 with fp32 PSUM accumulation
is numerically exact.

Per core, 2 batches of 4 images; SBUF partition layout [128 = (4 img, 32 ch)].
The three convs are FUSED per strip of R output rows with halo recompute
(strip s computes R+4 rows of h0 -> R+2 rows of h1 -> R output rows), so h0
and h1 never leave SBUF; only the padded sign(x) plane stages through DRAM.

 - conv0 (1->32ch): all 9 taps packed into K: input replicated into 36
   partitions (tap, img) via 9 shifted DMAs from the extended-pad sign(x)
   plane; one matmul per output row-pair (K=36, M=128, N=512).
 - conv1 (32->32ch): fp8 DoubleRow matmuls, 2 taps per pass: K=128 partitions
   x 2 k-subtiles; the rhs k-subtile offset is a free-dim shift on the padded
   input tile (4D AP), so 9 taps cost 5 passes instead of 9. Tap pairs share
   one (row,col) shift delta; the odd 9th tap is paired with zero weights.
 - conv2 (32->1ch): same DoubleRow pairing, M=4 (one column per image),
   psum [4, 8, 256] -> bf16 output (conv2 sums are even integers <= 288,
   exact in bf16).
PSUM is used as [128, 8, 256] 4-bank supertiles (4 row-pair matmul groups +
one batched eviction). sign() eviction: psum -> fp8 via ScalarE Sign (conv0)
and VectorE clamp(-1,1) (conv1; sums are integers so clamp == sign).
Issue order is software-pipelined (A of strip s+1 before B of strip s) to
hide eviction tails from the in-order PE queue.
"""

import numpy as np
import ml_dtypes

import concourse.bass as bass
import concourse.mybir as mybir
import concourse.tile as tile
from concourse import bacc
from concourse.bass_utils import run_bass_kernel_spmd

FP8 = mybir.dt.float8e4
BF16 = mybir.dt.bfloat16
F32 = mybir.dt.float32
AF = mybir.ActivationFunctionType
ALU = mybir.AluOpType
DRM = mybir.MatmulPerfMode.DoubleRow

N_CORES = 8
IMG_PER_CORE = 8
B = 4          # images per partition-batch
H = W = 256
WP = 258       # padded width (1 col pad each side)
HE = 262       # extended padded height: row = x row + 3
R = 64         # strip rows (output rows per strip)
NS = H // R    # strips per batch
NB = IMG_PER_CORE // B  # batches per core

# DoubleRow tap pairs: both taps of a pair share one flat shift delta
# (dy*WP + dx); the 9th tap is paired with zero weights (k-slot 1 unused).
PAIRS = [
    ((0, 0), (0, 1)),
    ((1, 0), (1, 1)),
    ((2, 0), (2, 1)),
    ((0, 2), (1, 2)),
    ((2, 2), None),
]


def _dr_rhs(hin, r, pair):
    """4D DoubleRow rhs AP: [128, ksub=2, rows=2, cols=256] with the ksub
    dim stepping by the tap-pair's shift delta over the padded tile."""
    (dy0, dx0), t1 = pair
    # the zero-weight dummy slot points one row up: always inside the tile
    delta = -WP if t1 is None else (t1[0] - dy0) * WP + (t1[1] - dx0)
    sl = hin[:, r + dy0:r + dy0 + 2, dx0:dx0 + 256]
    return bass.AP(
        tensor=sl.tensor, offset=sl.offset,
        ap=[list(sl.ap[0]), [delta, 2], list(sl.ap[1]), list(sl.ap[2])])


def _build_program(stages=('0', 'A', 'B', 'C')):
    nc = bacc.Bacc("TRN2", target_bir_lowering=False, debug=False)

    x_in = nc.dram_tensor("x", [IMG_PER_CORE, H, W], F32, kind="ExternalInput")
    s0_in = nc.dram_tensor("s0", [36, 128], FP8, kind="ExternalInput")
    s1_in = nc.dram_tensor("s1", [128, 5, 2, 128], FP8, kind="ExternalInput")
    s2_in = nc.dram_tensor("s2", [128, 5, 2, 16], FP8, kind="ExternalInput")
    out_d = nc.dram_tensor("out", [IMG_PER_CORE, H, W], BF16,
                           kind="ExternalOutput")

    # extended-pad sign(x): row = x row + 3 (rows 0-2 and 259-261 zero)
    xs_d = nc.dram_tensor("xs", [IMG_PER_CORE, HE, WP], FP8)

    with tile.TileContext(nc) as tc:
        with (
            tc.tile_pool(name="const", bufs=1) as cpool,
            tc.tile_pool(name="xprep", bufs=4) as xpool,
            tc.tile_pool(name="xrep", bufs=2) as xrpool,
            tc.tile_pool(name="h0", bufs=2) as h0pool,
            tc.tile_pool(name="h1", bufs=2) as h1pool,
            tc.tile_pool(name="cout", bufs=2) as cpool2,
            tc.tile_pool(name="psum", bufs=2, space="PSUM") as pspool,
        ):
            # --- constants: stationary weights + a zero tile ---
            s0t = cpool.tile([36, 128], FP8, tag="s0")
            nc.sync.dma_start(out=s0t[:, :], in_=s0_in[:, :])
            s1t = cpool.tile([128, 5, 2, 128], FP8, tag="s1")
            nc.sync.dma_start(out=s1t[:, :, :, :], in_=s1_in[:, :, :, :])
            s2t = cpool.tile([128, 5, 2, 16], FP8, tag="s2")
            nc.sync.dma_start(out=s2t[:, :, :, :], in_=s2_in[:, :, :, :])
            zt = cpool.tile([128, WP], FP8, tag="zt")
            nc.gpsimd.memset(zt[:, :], 0.0)

            # --- pre-zero xs pad rows (cols are baked into SBUF tiles) ---
            for img in range(IMG_PER_CORE):
                nc.scalar.dma_start(out=xs_d[img, 0:3, :], in_=zt[0:3, :])
                nc.scalar.dma_start(out=xs_d[img, HE - 3:HE, :],
                                    in_=zt[0:3, :])

            def stage_0(b):
                """sign(x) -> extended-pad fp8 planes in DRAM, batch b."""
                for img in range(b * B, (b + 1) * B):
                    for rb in range(H // 128):
                        xf = xpool.tile([128, W], F32, tag="xf")
                        nc.sync.dma_start(
                            out=xf[:, :],
                            in_=x_in[img, rb * 128:(rb + 1) * 128, :])
                        xp = xpool.tile([128, WP], FP8, tag="xp")
                        nc.scalar.activation(xp[:, 1:W + 1], xf[:, :], AF.Sign)
                        nc.gpsimd.memset(xp[:, 0:1], 0.0)
                        nc.gpsimd.memset(xp[:, WP - 1:WP], 0.0)
                        nc.gpsimd.dma_start(
                            out=xs_d[img, rb * 128 + 3:(rb + 1) * 128 + 3, :],
                            in_=xp[:, :])

            if '0' in stages:
                stage_0(0)

            for b in range(NB):
                # batch b+1's input prep hides under batch b's compute
                if '0' in stages and b + 1 < NB:
                    stage_0(b + 1)
                ht0s, ht1s = {}, {}

                def stage_a_units(s, b=b):
                    """conv0 strip s: h0 rows [sR-2, sR+R+2) -> ht0 tile
                    (tile row i = h0 row sR-2+i). Returns (ht0, units)."""
                    xt = xrpool.tile([36, R + 4, 256], FP8, tag="xrep",
                                     name="xt")
                    for dy in range(3):
                        for dx in range(3):
                            t = dy * 3 + dx
                            nc.sync.dma_start(
                                out=xt[4 * t:4 * t + 4, :, :],
                                in_=xs_d[b * B:(b + 1) * B,
                                         s * R + dy:s * R + dy + R + 4,
                                         dx:dx + 256])
                    ht0 = h0pool.tile([128, R + 4, WP], FP8, tag="h0",
                                      name="ht0")
                    nc.gpsimd.memset(ht0[:, :, 0:1], 0.0)
                    nc.gpsimd.memset(ht0[:, :, WP - 1:WP], 0.0)

                    def unit(r0, last):
                        nrow = min(8, R + 4 - r0)
                        ps = pspool.tile([128, 8, 256], F32, tag="ps",
                                         name="psA")
                        for q in range(nrow // 2):
                            nc.tensor.matmul(
                                ps[:, 2 * q:2 * q + 2, :], s0t[:, :],
                                xt[:, r0 + 2 * q:r0 + 2 * q + 2, :],
                                start=True, stop=True)
                        nc.scalar.activation(
                            ht0[:, r0:r0 + nrow, 1:W + 1],
                            ps[:, 0:nrow, :], AF.Sign)
                        if last:
                            # boundary: h0 pad rows (-1 / 256) must be zero
                            if s == 0:
                                nc.gpsimd.memset(ht0[:, 1:2, :], 0.0)
                            if s == NS - 1:
                                nc.gpsimd.memset(ht0[:, R + 2:R + 3, :], 0.0)

                    r0s = list(range(0, R + 4, 8))
                    return ht0, [
                        (lambda r0=r0, last=(r0 == r0s[-1]): unit(r0, last))
                        for r0 in r0s]

                def stage_b_units(s, ht0):
                    """conv1 strip s: h1 rows [sR-1, sR+R+1) -> ht1 tile
                    (tile row i = h1 row sR-1+i); input ht0."""
                    ht1 = h1pool.tile([128, R + 2, WP], FP8, tag="h1",
                                      name="ht1")
                    nc.gpsimd.memset(ht1[:, :, 0:1], 0.0)
                    nc.gpsimd.memset(ht1[:, :, WP - 1:WP], 0.0)

                    def unit(r0, last):
                        nrow = min(8, R + 2 - r0)
                        ps = pspool.tile([128, 8, 256], F32, tag="ps",
                                         name="psB")
                        for q in range(nrow // 2):
                            for t, pair in enumerate(PAIRS):
                                nc.tensor.matmul(
                                    ps[:, 2 * q:2 * q + 2, :], s1t[:, t, :, :],
                                    _dr_rhs(ht0, r0 + 2 * q, pair),
                                    start=(t == 0), stop=(t == 4),
                                    perf_mode=DRM)
                        # alternate eviction engines to keep the psum
                        # rotation fed (clamp(-1,1) == sign for int sums)
                        if (r0 // 8) % 2 == 0:
                            nc.scalar.activation(
                                ht1[:, r0:r0 + nrow, 1:W + 1],
                                ps[:, 0:nrow, :], AF.Sign)
                        else:
                            nc.vector.tensor_scalar(
                                ht1[:, r0:r0 + nrow, 1:W + 1],
                                ps[:, 0:nrow, :], -1.0, 1.0,
                                ALU.max, ALU.min)
                        if last:
                            # boundary: h1 pad rows (-1 / 256) must be zero
                            if s == 0:
                                nc.gpsimd.memset(ht1[:, 0:1, :], 0.0)
                            if s == NS - 1:
                                nc.gpsimd.memset(ht1[:, R + 1:R + 2, :], 0.0)

                    r0s = list(range(0, R + 2, 8))
                    return ht1, [
                        (lambda r0=r0, last=(r0 == r0s[-1]): unit(r0, last))
                        for r0 in r0s]

                def stage_c_units(s, ht1, b=b):
                    """conv2 strip s: out rows [sR, sR+R); input ht1."""
                    ot = cpool2.tile([B, R, W], BF16, tag="c_out", name="ot")

                    def unit(r0, last):
                        ps = pspool.tile([128, 8, 256], F32, tag="ps",
                                         name="psC")
                        for q in range(4):
                            for t, pair in enumerate(PAIRS):
                                nc.tensor.matmul(
                                    ps[0:B, 2 * q:2 * q + 2, :],
                                    s2t[:, t, :, 0:B],
                                    _dr_rhs(ht1, r0 + 2 * q, pair),
                                    start=(t == 0), stop=(t == 4),
                                    perf_mode=DRM)
                        nc.vector.tensor_copy(
                            ot[:, r0:r0 + 8, :], ps[0:B, :, :])
                        if last:
                            for g in range(B):
                                nc.gpsimd.dma_start(
                                    out=out_d[b * B + g, s * R:s * R + R, :],
                                    in_=ot[g:g + 1, :, :])

                    r0s = list(range(0, R, 8))
                    return [
                        (lambda r0=r0, last=(r0 == r0s[-1]): unit(r0, last))
                        for r0 in r0s]

                # software pipeline, depth 2, interleaved at supertile
                # granularity: fast-matmul A units ride alongside slow-matmul
                # B/C units so the in-order PE never drains the 2-buffer PSUM
                # rotation waiting on an eviction.
                for s in range(NS + 2):
                    units = []
                    if s < NS and 'A' in stages:
                        ht0s[s], ua = stage_a_units(s)
                        units.append(ua)
                    if 0 <= s - 1 < NS and 'B' in stages and s - 1 in ht0s:
                        ht1s[s - 1], ub = stage_b_units(s - 1,
                                                        ht0s.pop(s - 1))
                        units.append(ub)
                    if 0 <= s - 2 < NS and 'C' in stages and s - 2 in ht1s:
                        units.append(stage_c_units(s - 2, ht1s.pop(s - 2)))
                    for i in range(max(map(len, units), default=0)):
                        for u in units:
                            if i < len(u):
                                u[i]()
    nc.compile()
    return nc


def _host_weights(w0, w1, w2):
    """Pack sign(w) into fp8 stationary matrices (see module docstring)."""
    f8 = ml_dtypes.float8_e4m3
    sg = lambda w: np.sign(np.asarray(w, np.float32))
    w0s, w1s, w2s = sg(w0), sg(w1), sg(w2)  # [32,1,3,3],[32,32,3,3],[1,32,3,3]
    s0 = np.zeros((36, 128), np.float32)
    s1 = np.zeros((128, 5, 2, 128), np.float32)
    s2 = np.zeros((128, 5, 2, 16), np.float32)
    for g in range(B):
        for dy in range(3):
            for dx in range(3):
                t = dy * 3 + dx
                s0[t * 4 + g, g * 32:(g + 1) * 32] = w0s[:, 0, dy, dx]
        for t, ((dy0, dx0), t1) in enumerate(PAIRS):
            s1[g * 32:(g + 1) * 32, t, 0, g * 32:(g + 1) * 32] = \
                w1s[:, :, dy0, dx0].T  # [ci, co]
            s2[g * 32:(g + 1) * 32, t, 0, g] = w2s[0, :, dy0, dx0]
            if t1 is not None:
                s1[g * 32:(g + 1) * 32, t, 1, g * 32:(g + 1) * 32] = \
                    w1s[:, :, t1[0], t1[1]].T
                s2[g * 32:(g + 1) * 32, t, 1, g] = w2s[0, :, t1[0], t1[1]]
    return s0.astype(f8), s1.astype(f8), s2.astype(f8)


_NC_CACHE = {}


def kernel(x, w0, w1, w2):
    if "nc" not in _NC_CACHE:
        _NC_CACHE["nc"] = _build_program()
    nc = _NC_CACHE["nc"]
    s0, s1, s2 = _host_weights(w0, w1, w2)
    x = np.asarray(x, np.float32).reshape(64, H, W)
    in_maps = [
        {"x": np.ascontiguousarray(x[i * IMG_PER_CORE:(i + 1) * IMG_PER_CORE]),
         "s0": s0, "s1": s1, "s2": s2}
        for i in range(N_CORES)
    ]
    res = run_bass_kernel_spmd(nc, in_maps, list(range(N_CORES)))
    out = np.stack([np.asarray(res.results[i]["out"], np.float32)
                    for i in range(N_CORES)])
    return out.reshape(64, 1, H, W)
